# revision 20
# baseline (speedup 1.0000x reference)
"""Trainium2 Bass kernel for a dense transformer block.

Block: y = x + proj(MHA(LN1(x), rel-pos-bias)) ; out = y + fc2(gelu(fc1(LN2(y))))
Shapes (hardcoded): B=4, N=2048, C=512, H=8, DH=64, HID=2048, fp32 I/O.

Sharding over 8 cores: core c -> (batch b = c//2, query-half par = c%2).
Each core receives its batch's rows rolled so its own 1024 query tokens come
first, computes K/V over all 2048 tokens (duplicated across the pair of cores
sharing a batch -- cheaper than a collective), and runs attention + MLP for its
own 1024 tokens. Weights are replicated; LayerNorm affine params are folded
into the matmul weights on the host.

Engine-balance design (v2):
  - softmax bias enters MULTIPLICATIVELY: exp(s+b) = exp(s)*exp(b).  The ACT
    engine exps score PSUM directly ([128,2048] spanning 4 banks covers both
    heads of a pair in one instruction); host supplies exp(bias) blocks in
    bf16; the bias application is then a bf16*bf16 SBUF DVE multiply which
    runs in the DVE's 4x perf mode.
  - LN rsqrt = exp(-0.5*ln(var+eps)) with var columns batched across tiles,
    so the only ACT table sets used are natural_log_exp (A/C/D) and gelu (E).
  - fc1 bias rides the Gelu activation's per-partition bias operand (ACT
    evicts the fc1 PSUM directly); the V bias is folded through the
    attention-average into proj_b on the host; fc2/proj biases enter via a
    K=2 ones-matmul with hi+lo bf16 rows.
  - scores matmuls are head-paired via PE row tiling (K=64 each, partitions
    0-63 / 64-127 -> tile_position (0,0)/(64,0) auto-derived), attnV uses the
    ones-augmented V (M=65) so the softmax denominator accumulates in row 64.
  - O^T stays in SBUF (no DRAM roundtrip); transpose evictions are merged
    into single 512-wide strided copies.
"""

import threading
from contextlib import ExitStack

import numpy as np

import concourse.bass as bass
import concourse.tile as tile
from concourse import bacc, mybir
from concourse.bass_utils import run_bass_kernel_spmd
from concourse.masks import make_identity

F32 = mybir.dt.float32
BF16 = mybir.dt.bfloat16
FP8 = mybir.dt.float8e4

B, N, C, H = 4, 2048, 512, 8
DH = C // H          # 64
HID = 4 * C          # 2048
NQ = N // 2          # own query tokens per core (1024)
EPS = 1e-5
P = 128              # partitions
TT = N // P          # 16 token tiles (full batch)
TQ = NQ // P         # 8 token tiles (own)
CT = C // P          # 4 channel tiles
OT = HID // P        # 16 hidden tiles
BLKW = NQ + 7 * P    # 1920, bias block width


def build_program(reps: int = 1, phases: str = "abcde"):
    """Build the per-core Bass program (SPMD; all per-core differences are
    carried by input data)."""
    nc = bacc.Bacc("TRN2", target_bir_lowering=False, debug=False, num_devices=8)

    t = {}
    t["xb"] = nc.dram_tensor("xb", [N, C], F32, kind="ExternalInput").ap()
    t["wqkvT"] = nc.dram_tensor("wqkvT", [C, 3 * C], BF16,
                                kind="ExternalInput").ap()
    t["bqk"] = nc.dram_tensor("bqk", [2 * C], F32, kind="ExternalInput").ap()
    t["wprojT"] = nc.dram_tensor("wprojT", [C, C], BF16,
                                 kind="ExternalInput").ap()
    t["bproj2"] = nc.dram_tensor("bproj2", [2, C], BF16,
                                 kind="ExternalInput").ap()
    t["wfc1T"] = nc.dram_tensor("wfc1T", [C, HID], BF16,
                                kind="ExternalInput").ap()
    t["bfc1"] = nc.dram_tensor("bfc1", [HID], F32, kind="ExternalInput").ap()
    t["wfc2T"] = nc.dram_tensor("wfc2T", [HID, C], BF16,
                                kind="ExternalInput").ap()
    t["bfc22"] = nc.dram_tensor("bfc22", [2, C], BF16,
                                kind="ExternalInput").ap()
    t["eblka"] = nc.dram_tensor("eblka", [H, P, BLKW], BF16,
                                kind="ExternalInput").ap()
    t["eblkb"] = nc.dram_tensor("eblkb", [H, P, BLKW], BF16,
                                kind="ExternalInput").ap()
    t["out"] = nc.dram_tensor("out", [NQ, C], F32, kind="ExternalOutput").ap()

    with tile.TileContext(nc) as tc:
        if reps == 1:
            _build_body(nc, tc, t)
        else:
            with tc.For_i(0, reps, 1):
                _build_body(nc, tc, t)
    nc.compile()
    return nc


def _build_body(nc, tc, t):
    Act = mybir.ActivationFunctionType
    Alu = mybir.AluOpType

    xb, out = t["xb"], t["out"]

    with ExitStack() as ctx:
        singles = ctx.enter_context(tc.tile_pool(name="singles", bufs=1))
        ident = singles.tile([P, P], F32)
        make_identity(nc, ident)
        identB = singles.tile([P, P], BF16)
        nc.vector.tensor_copy(out=identB, in_=ident)
        eps_t = singles.tile([P, 1], F32)
        nc.gpsimd.memset(eps_t, EPS)
        ones2 = singles.tile([2, P], BF16)
        nc.gpsimd.memset(ones2, 1.0)
        ones64f = singles.tile([1, DH], F32)
        nc.gpsimd.memset(ones64f, 1.0)

        x_all = [None] * TT
        kT8 = [None] * 2     # [P, 2*N] fp8: heads 4g+j at parts 32j, dh-half
        qT8 = [None] * 2     # [P, 2*NQ] fp8, same packing
        kTh = [None] * H     # [32, 2*N] fp8 per head at partitions 0:32
        qTh = [None] * H     # [32, 2*NQ] fp8 per head (DR tiles at (0,0):
        # nonzero-row-position DoubleRow matmuls fault on TRN2 hw, so the
        # packed evictions are DMA-remapped down to partition 0 per head)
        va = [None] * TT

        xq_pool = ctx.enter_context(tc.tile_pool(name="xq", bufs=TQ))
        oT_pool = ctx.enter_context(tc.tile_pool(name="oT", bufs=CT))
        # D/E weights live here so they can prefetch during phase C while
        # phase-C pools (created later) still release first (LIFO).
        wts_pool = ctx.enter_context(tc.tile_pool(name="wts", bufs=1))
        ac_scope = ctx.enter_context(ExitStack())  # spans phases A..C
        kT_pool = ac_scope.enter_context(tc.tile_pool(name="kT", bufs=CT))
        qT_pool = ac_scope.enter_context(tc.tile_pool(name="qT", bufs=CT))
        va_pool = ac_scope.enter_context(tc.tile_pool(name="va", bufs=TT))
        # exp-bias block pool outlives the A/B scope (prefetched during B)
        eb_pool = ac_scope.enter_context(tc.tile_pool(name="eb", bufs=4))

        # O^T in SBUF: oT[hp] is [128, NQ] holding heads 2hp (rows 0:64) and
        # 2hp+1 (rows 64:128) -- exactly the proj lhsT channel tile.
        oT = [oT_pool.tile([P, NQ], BF16, tag="oT", name=f"oT{i}")
              for i in range(CT)]

        # ------------------------------------------------------------------
        # Phases A+B+C, software-pipelined: LN1 is processed in two token
        # halves; QKV chunks for heads 2.. are emitted inside the attention
        # kt-loops of earlier head-pairs (PE has slack under the exp period).
        # ------------------------------------------------------------------
        ab = ac_scope.enter_context(ExitStack())
        z1t_pool = ab.enter_context(tc.tile_pool(name="z1t", bufs=1))
        xload_pool = ab.enter_context(
            tc.tile_pool(name="xload", bufs=TT - TQ))
        zt_pool = ab.enter_context(tc.tile_pool(name="zt", bufs=3))
        stat_pool = ab.enter_context(tc.tile_pool(name="stat", bufs=4))
        mv_pool = ab.enter_context(tc.tile_pool(name="mv1", bufs=1))
        wq_pool = ab.enter_context(tc.tile_pool(name="wq", bufs=CT))
        bias_pool = ab.enter_context(tc.tile_pool(name="qkvb", bufs=1))

        # z1t split into token halves so QKV can start after half A
        z1h = []      # z1h[half][ct] = [P, NQ] view
        z1c = []
        for half in range(2):
            z_all = z1t_pool.tile([P, CT * NQ], BF16, tag=f"z1t{half}",
                                  name=f"z1t{half}")
            z1h.append([z_all[:, ct * NQ:(ct + 1) * NQ] for ct in range(CT)])
            z1c.append(z_all.rearrange("p (c n) -> p c n", c=CT))

        mv_all = mv_pool.tile([P, 2 * TT], F32, tag="mv")
        lnv = mv_pool.tile([P, TT], F32, tag="lnv")
        rs_all = mv_pool.tile([P, TT], F32, tag="rs")
        mv_t = mv_all.rearrange("p (t two) -> p t two", two=2)
        lnv_t = lnv.rearrange("p (t one) -> p t one", one=1)

        with ExitStack() as abp:
            tpsum = abp.enter_context(
                tc.tile_pool(name="tpsum", bufs=2, space="PSUM"))
            bpsum = abp.enter_context(
                tc.tile_pool(name="bpsum", bufs=4, space="PSUM"))

            def ln1_quarter(qb):
                t0, t1 = qb * 4, qb * 4 + 4
                for tt in range(t0, t1):
                    if tt < TQ:
                        x_t = xq_pool.tile([P, C], F32, tag="xq")
                    else:
                        x_t = xload_pool.tile([P, C], F32, tag="xload")
                    x_all[tt] = x_t
                    nc.sync.dma_start(out=x_t, in_=xb[tt * P:(tt + 1) * P, :])
                    st = stat_pool.tile([P, 6], F32, tag="st")
                    nc.vector.bn_stats(out=st, in_=x_t)
                    nc.vector.bn_aggr(out=mv_all[:, 2 * tt:2 * tt + 2],
                                      in_=st)
                # rs = exp(-0.5*ln(var+eps)), batched over the half-batch
                nc.scalar.activation(out=lnv_t[:, t0:t1, :],
                                     in_=mv_t[:, t0:t1, 1:2],
                                     func=Act.Ln, bias=eps_t, scale=1.0)
                nc.scalar.activation(out=rs_all[:, t0:t1],
                                     in_=lnv[:, t0:t1],
                                     func=Act.Exp, scale=-0.5)
                for tt in range(t0, t1):
                    z_t = zt_pool.tile([P, C], BF16, tag="zt")
                    nc.vector.tensor_scalar(
                        out=z_t, in0=x_all[tt],
                        scalar1=mv_all[:, 2 * tt:2 * tt + 1],
                        scalar2=rs_all[:, tt:tt + 1],
                        op0=Alu.subtract, op1=Alu.mult)
                    ps4 = tpsum.tile([P, C], BF16, tag="tr")
                    for ct in range(CT):
                        nc.tensor.transpose(
                            ps4[:, ct * P:(ct + 1) * P],
                            z_t[:, ct * P:(ct + 1) * P], identB)
                    hb, tl = tt // 8, tt % 8
                    nc.vector.tensor_copy(
                        out=z1c[hb][:, :, tl * P:(tl + 1) * P],
                        in_=ps4.rearrange("p (c n) -> p c n", c=CT))

            wsb = []
            bcols = []

            def qkv_weights():
                for g in range(2):
                    kT8[g] = kT_pool.tile([P, 2 * N], FP8, tag="kT",
                                          name=f"kT{g}", bufs=2)
                    qT8[g] = qT_pool.tile([P, 2 * NQ], FP8, tag="qT",
                                          name=f"qT{g}", bufs=2)
                for ct in range(CT):
                    w_t = wq_pool.tile([P, 3 * C], BF16, tag="wq")
                    nc.sync.dma_start(
                        out=w_t, in_=t["wqkvT"][ct * P:(ct + 1) * P, :])
                    wsb.append(w_t)
                for ot in range(8):
                    bt = bias_pool.tile([P, 1], F32, tag="bcol", bufs=8)
                    nc.sync.dma_start(
                        out=bt,
                        in_=t["bqk"][ot * P:(ot + 1) * P].rearrange(
                            "(p one) -> p one", one=1))
                    bcols.append(bt)

            def v_tile(tt):
                # V natural [tok, 512] + ones column per head -> [P, H, 65]
                hb, tl = tt // 8, tt % 8
                v_t = va_pool.tile([P, H * (DH + 1)], BF16, tag="va")
                va[tt] = v_t
                nc.gpsimd.memset(v_t, 1.0)
                ps = bpsum.tile([P, 512], F32, tag="mm")
                for ct in range(CT):
                    nc.tensor.matmul(
                        ps,
                        lhsT=z1h[hb][ct][:, tl * P:(tl + 1) * P],
                        rhs=wsb[ct][:, 2 * C:3 * C],
                        start=(ct == 0), stop=(ct == CT - 1))
                nc.vector.tensor_copy(
                    out=v_t.rearrange("p (h w) -> p h w",
                                      w=DH + 1)[:, :, 0:DH],
                    in_=ps.rearrange("p (h w) -> p h w", w=DH))

            def k_chunk(ot, tch):
                # K^T o-tile ot=(g,i), 512 tokens at tch*512; fp8 eviction
                # into the DoubleRow dh-split layout.
                g, i = ot // 2, ot % 2
                hb, tl = tch // 2, tch % 2
                ps = bpsum.tile([P, 512], F32, tag="mm")
                for ct in range(CT):
                    nc.tensor.matmul(
                        ps,
                        lhsT=wsb[ct][:, C + ot * P:C + (ot + 1) * P],
                        rhs=z1h[hb][ct][:, tl * 512:(tl + 1) * 512],
                        start=(ct == 0), stop=(ct == CT - 1))
                nc.vector.tensor_scalar_add(
                    out=kT8[g][:, i * N + tch * 512:i * N + (tch + 1) * 512],
                    in0=ps, scalar1=bcols[4 + ot])

            def q_chunk(ot, tch):
                # Q^T o-tile ot=(g,i), own tokens only (token half A)
                g, i = ot // 2, ot % 2
                ps = bpsum.tile([P, 512], F32, tag="mm")
                for ct in range(CT):
                    nc.tensor.matmul(
                        ps,
                        lhsT=wsb[ct][:, ot * P:(ot + 1) * P],
                        rhs=z1h[0][ct][:, tch * 512:(tch + 1) * 512],
                        start=(ct == 0), stop=(ct == CT - 1))
                nc.vector.tensor_scalar_add(
                    out=qT8[g][:, i * NQ + tch * 512:i * NQ + (tch + 1) * 512],
                    in0=ps, scalar1=bcols[ot])

            def qkv_head_chunks(ot):
                return ([("k", ot, tch) for tch in range(4)]
                        + [("q", ot, tch) for tch in range(2)])

            def emit_chunk(c):
                kind = c[0]
                if kind == "k":
                    k_chunk(c[1], c[2])
                elif kind == "q":
                    q_chunk(c[1], c[2])
                elif kind == "v":
                    v_tile(c[1])

            # exp-bias block loads ride the ACT hwdge queue so they stream
            # in parallel with the x/weight loads on the SP queue.
            eb_all = [None] * H

            def eb_load(h):
                bb = eb_pool.tile([P, 2 * BLKW], BF16, tag="eb")
                nc.scalar.dma_start(out=bb[:, 0:BLKW], in_=t["eblka"][h])
                nc.scalar.dma_start(out=bb[:, BLKW:2 * BLKW],
                                    in_=t["eblkb"][h])
                return bb

            # --- emission: A quarters; heads-0-3 QKV first; V; rest ---
            ln1_quarter(0)
            ln1_quarter(1)
            qkv_weights()
            for ot in range(2):
                for tch in range(2):
                    k_chunk(ot, tch)
            for ot in range(2):
                for tch in range(2):
                    q_chunk(ot, tch)
            for tt in range(8):
                v_tile(tt)
            ln1_quarter(2)
            ln1_quarter(3)
            eb_all[0] = eb_load(0)
            for ot in range(2):
                for tch in range(2, 4):
                    k_chunk(ot, tch)
            for tt in range(8, 16):
                v_tile(tt)
            eb_all[1] = eb_load(1)

            def head_remap(h):
                g, j = h // 4, h % 4
                kTh[h] = kT_pool.tile([32, 2 * N], FP8, tag="kTh",
                                      name=f"kTh{h}", bufs=4)
                qTh[h] = qT_pool.tile([32, 2 * NQ], FP8, tag="qTh",
                                      name=f"qTh{h}", bufs=4)
                nc.sync.dma_start(out=kTh[h],
                                  in_=kT8[g][32 * j:32 * (j + 1), :])
                nc.sync.dma_start(out=qTh[h],
                                  in_=qT8[g][32 * j:32 * (j + 1), :])

            for h in range(4):
                head_remap(h)
            # remaining QKV o-tiles (heads 4-7; PSUM is full during
            # attention, so these are emitted up front)
            for ot in range(2, 4):
                for tch in range(4):
                    k_chunk(ot, tch)
                for tch in range(2):
                    q_chunk(ot, tch)
            for h in range(4, H):
                head_remap(h)
            # D/E weight prefetch on the (now idle) SP queue
            wpsb = []
            for ct in range(CT):
                w_t = wts_pool.tile([P, C], BF16, tag="wp", bufs=CT)
                nc.sync.dma_start(out=w_t,
                                  in_=t["wprojT"][ct * P:(ct + 1) * P, :])
                wpsb.append(w_t)
            bprow = wts_pool.tile([2, C], BF16, tag="bprow")
            nc.sync.dma_start(out=bprow, in_=t["bproj2"])
            w1sb = []
            for ct in range(CT):
                w_t = wts_pool.tile([P, HID], BF16, tag="w1", bufs=CT)
                nc.sync.dma_start(out=w_t,
                                  in_=t["wfc1T"][ct * P:(ct + 1) * P, :])
                w1sb.append(w_t)
            w2sb = []
            for ot in range(OT):
                w_t = wts_pool.tile([P, C], BF16, tag="w2", bufs=OT)
                nc.sync.dma_start(out=w_t,
                                  in_=t["wfc2T"][ot * P:(ot + 1) * P, :])
                w2sb.append(w_t)
            b1cols = []
            for ot in range(OT):
                bt = wts_pool.tile([P, 1], F32, tag="b1c", bufs=OT)
                nc.sync.dma_start(
                    out=bt, in_=t["bfc1"][ot * P:(ot + 1) * P].rearrange(
                        "(p one) -> p one", one=1))
                b1cols.append(bt)
            b2row = wts_pool.tile([2, C], BF16, tag="b2row")
            nc.sync.dma_start(out=b2row, in_=t["bfc22"])
            abp.close()   # free tpsum/bpsum banks for attention
            ab.close()    # free z1t/xload/wq SBUF before attention tiles

            # --------------------------------------------------------------
            # Phase C: attention, head-pairs, flash-style over k tiles.
            # Per (kt, head): scores -> one [128,1024] 2-bank PSUM chunk,
            # one Exp -> bf16, one 4x-mode DVE multiply by exp(bias).
            # attnV uses ones-augmented V (M=65): denominator in row 64.
            # The 1/den broadcast lands in partitions 64:128 of the (full
            # height) ac tile, so normalize needs no extra PSUM.
            # --------------------------------------------------------------
            cx = ac_scope.enter_context(ExitStack())
            texp_pool = cx.enter_context(tc.tile_pool(name="texp", bufs=4))
            traw_pool = cx.enter_context(tc.tile_pool(name="traw", bufs=3))
            rden_pool = cx.enter_context(tc.tile_pool(name="rden", bufs=2))
            scp = cx.enter_context(
                tc.tile_pool(name="scp", bufs=2, space="PSUM"))
            acp = cx.enter_context(
                tc.tile_pool(name="acp", bufs=2, space="PSUM"))

            def normalize(h, ac):
                # O^T = num * (1/den); den is row 64 of ac; 1/den broadcast
                # via fp32 rank-1 matmul into rows 64:128 of ac itself, then
                # staged to SBUF bf16 (DVE may read only one PSUM operand).
                den_sb = rden_pool.tile([1, NQ], F32, tag="densb", bufs=2)
                nc.vector.tensor_copy(out=den_sb, in_=ac[DH:DH + 1, :])
                rden = rden_pool.tile([1, NQ], F32, tag="rden")
                nc.vector.reciprocal_approx_fast(out=rden, in_=den_sb)
                for qc in range(2):
                    sl = slice(qc * 512, (qc + 1) * 512)
                    nc.tensor.matmul(ac[DH:2 * DH, sl],
                                     lhsT=ones64f, rhs=rden[:, sl],
                                     start=True, stop=True)
                pb_sb = rden_pool.tile([DH, NQ], BF16, tag="pbsb", bufs=2)
                nc.vector.tensor_copy(out=pb_sb, in_=ac[DH:2 * DH, :])
                nc.vector.tensor_tensor(
                    out=oT[h // 2][(h % 2) * DH:(h % 2 + 1) * DH, :],
                    in0=ac[0:DH, :], in1=pb_sb, op=Alu.mult)

            DRmode = mybir.MatmulPerfMode.DoubleRow
            pend_norm = None
            for h in range(H):
                g, j = h // 4, h % 4
                if h + 2 < H:
                    eb_all[h + 2] = eb_load(h + 2)
                bb = eb_all[h]
                kTv = kTh[h].rearrange("p (two n) -> p two n", two=2)
                qTv = qTh[h].rearrange("p (two n) -> p two n", two=2)
                ac = acp.tile([P, NQ], F32, tag="ac", name=f"ac{h}")
                pend = None
                for kt in range(TT):
                    off = (0 if kt < 8 else BLKW) + (7 - kt % 8) * P
                    scl = scp.tile([P, NQ], F32, tag="sc",
                                   name=f"sc{h}_{kt}")
                    for qc in range(2):
                        nc.tensor.matmul(
                            scl[:, qc * 512:(qc + 1) * 512],
                            lhsT=kTv[:, :, kt * P:(kt + 1) * P],
                            rhs=qTv[:, :, qc * 512:(qc + 1) * 512],
                            start=True, stop=True, perf_mode=DRmode,
                            tile_position=(0, 0))
                    traw = traw_pool.tile([P, NQ], BF16, tag="traw")
                    nc.scalar.activation(out=traw, in_=scl, func=Act.Exp,
                                         scale=float(DH) ** -0.5)
                    tx = texp_pool.tile([P, NQ], BF16, tag="tx",
                                        name=f"tx{h}_{kt}")
                    nc.vector.tensor_tensor(
                        out=tx, in0=traw, in1=bb[:, off:off + NQ],
                        op=Alu.mult)
                    if kt == 1 and pend_norm is not None:
                        normalize(*pend_norm)
                        pend_norm = None
                    if pend is not None:
                        ptx, pkt = pend
                        for qc in range(2):
                            nc.tensor.matmul(
                                ac[0:DH + 1, qc * 512:(qc + 1) * 512],
                                lhsT=va[pkt][:, h * (DH + 1):
                                             (h + 1) * (DH + 1)],
                                rhs=ptx[:, qc * 512:(qc + 1) * 512],
                                start=(pkt == 0), stop=False)
                    pend = (tx, kt)
                ptx, pkt = pend
                for qc in range(2):
                    nc.tensor.matmul(
                        ac[0:DH + 1, qc * 512:(qc + 1) * 512],
                        lhsT=va[pkt][:, h * (DH + 1):(h + 1) * (DH + 1)],
                        rhs=ptx[:, qc * 512:(qc + 1) * 512],
                        start=False, stop=True)
                pend_norm = (h, ac)
            normalize(*pend_norm)
        ac_scope.close()  # free kT/qT/va/z1t/eb/texp + attention PSUM

        # ------------------------------------------------------------------
        # Phase D: proj + residual + LN2 (transposed), two batches of 4
        # ------------------------------------------------------------------
        x2_pool = ctx.enter_context(tc.tile_pool(name="x2", bufs=TQ))
        z2t_pool = ctx.enter_context(tc.tile_pool(name="z2t", bufs=1))
        z2t_all = z2t_pool.tile([P, CT * NQ], BF16, tag="z2t")
        z2t = [z2t_all[:, ct * NQ:(ct + 1) * NQ] for ct in range(CT)]
        z2t_c = z2t_all.rearrange("p (c n) -> p c n", c=CT)
        x2 = [None] * TQ
        with ExitStack() as dx:
            stat2_pool = dx.enter_context(tc.tile_pool(name="stat2", bufs=4))
            mv2_pool = dx.enter_context(tc.tile_pool(name="mv2", bufs=1))
            zt2_pool = dx.enter_context(tc.tile_pool(name="zt2", bufs=3))
            dpsum = dx.enter_context(
                tc.tile_pool(name="dpsum", bufs=2, space="PSUM"))
            tpsum2 = dx.enter_context(
                tc.tile_pool(name="tpsum2", bufs=2, space="PSUM"))

            mv2_all = mv2_pool.tile([P, 2 * TQ], F32, tag="mv2")
            lnv2 = mv2_pool.tile([P, TQ], F32, tag="lnv2")
            rs2_all = mv2_pool.tile([P, TQ], F32, tag="rs2")
            mv2_t = mv2_all.rearrange("p (t two) -> p t two", two=2)
            lnv2_t = lnv2.rearrange("p (t one) -> p t one", one=1)

            for hb in range(2):
                t0, t1 = hb * 4, hb * 4 + 4
                for tq in range(t0, t1):
                    ps = dpsum.tile([P, C], F32, tag="mm")
                    for ct in range(CT):
                        nc.tensor.matmul(
                            ps, lhsT=oT[ct][:, tq * P:(tq + 1) * P],
                            rhs=wpsb[ct], start=(ct == 0), stop=False)
                    nc.tensor.matmul(ps, lhsT=ones2, rhs=bprow,
                                     start=False, stop=True)
                    x2_t = x2_pool.tile([P, C], F32, tag="x2")
                    x2[tq] = x2_t
                    nc.vector.tensor_add(out=x2_t, in0=ps, in1=x_all[tq])
                    st = stat2_pool.tile([P, 6], F32, tag="st2")
                    nc.vector.bn_stats(out=st, in_=x2_t)
                    nc.vector.bn_aggr(out=mv2_all[:, 2 * tq:2 * tq + 2],
                                      in_=st)
                nc.scalar.activation(out=lnv2_t[:, t0:t1, :],
                                     in_=mv2_t[:, t0:t1, 1:2],
                                     func=Act.Ln, bias=eps_t, scale=1.0)
                nc.scalar.activation(out=rs2_all[:, t0:t1],
                                     in_=lnv2[:, t0:t1],
                                     func=Act.Exp, scale=-0.5)
                for tq in range(t0, t1):
                    z_t = zt2_pool.tile([P, C], BF16, tag="z2tmp")
                    nc.vector.tensor_scalar(
                        out=z_t, in0=x2[tq],
                        scalar1=mv2_all[:, 2 * tq:2 * tq + 1],
                        scalar2=rs2_all[:, tq:tq + 1],
                        op0=Alu.subtract, op1=Alu.mult)
                    ps4 = tpsum2.tile([P, C], BF16, tag="tr2")
                    for ct in range(CT):
                        nc.tensor.transpose(
                            ps4[:, ct * P:(ct + 1) * P],
                            z_t[:, ct * P:(ct + 1) * P], identB)
                    nc.vector.tensor_copy(
                        out=z2t_c[:, :, tq * P:(tq + 1) * P],
                        in_=ps4.rearrange("p (c n) -> p c n", c=CT))

        # ------------------------------------------------------------------
        # Phase E: MLP.  fc1 bias rides the Gelu bias operand (ACT evicts
        # the fc1 PSUM); fc2 runs ot-outer into 6 resident accumulators so
        # its matmuls interleave with fc1's instead of waiting for all of
        # gelu -- only tq 6..7 run as a short tail.
        # ------------------------------------------------------------------
        with ExitStack() as ex:
            g_pool = ex.enter_context(tc.tile_pool(name="g", bufs=3))
            out_pool = ex.enter_context(tc.tile_pool(name="outp", bufs=2))
            gpsum = ex.enter_context(
                tc.tile_pool(name="gpsum", bufs=2, space="PSUM"))
            epsum = ex.enter_context(
                tc.tile_pool(name="epsum", bufs=6, space="PSUM"))

            NACC = 6

            def emit_out(tq, ps):
                nc.tensor.matmul(ps, lhsT=ones2, rhs=b2row,
                                 start=False, stop=True)
                o_t = out_pool.tile([P, C], F32, tag="out")
                nc.vector.tensor_add(out=o_t, in0=ps, in1=x2[tq])
                nc.sync.dma_start(out=out[tq * P:(tq + 1) * P, :], in_=o_t)

            eps_acc = [epsum.tile([P, C], F32, tag="mm2", name=f"fc2a{i}")
                       for i in range(NACC)]
            gT = []
            for ot in range(OT):
                g_t = g_pool.tile([P, NQ], BF16, tag="g", bufs=OT)
                gT.append(g_t)
                for qc in range(NQ // 512):
                    psg = gpsum.tile([P, 512], F32, tag="mm1")
                    for ct in range(CT):
                        nc.tensor.matmul(
                            psg,
                            lhsT=w1sb[ct][:, ot * P:(ot + 1) * P],
                            rhs=z2t[ct][:, qc * 512:(qc + 1) * 512],
                            start=(ct == 0), stop=(ct == CT - 1))
                    nc.scalar.activation(
                        out=g_t[:, qc * 512:(qc + 1) * 512], in_=psg,
                        func=Act.Gelu, bias=b1cols[ot], scale=1.0)
                # fc2 partials for tq 0..NACC-1 ride along (one ot behind)
                if ot >= 1:
                    for tq in range(NACC):
                        nc.tensor.matmul(
                            eps_acc[tq],
                            lhsT=gT[ot - 1][:, tq * P:(tq + 1) * P],
                            rhs=w2sb[ot - 1],
                            start=(ot - 1 == 0), stop=False)
            for tq in range(NACC):
                nc.tensor.matmul(
                    eps_acc[tq], lhsT=gT[OT - 1][:, tq * P:(tq + 1) * P],
                    rhs=w2sb[OT - 1], start=False, stop=False)
                emit_out(tq, eps_acc[tq])
            for tq in range(NACC, TQ):
                ps = epsum.tile([P, C], F32, tag="mm2", name=f"fc2t{tq}")
                for ot in range(OT):
                    nc.tensor.matmul(
                        ps, lhsT=gT[ot][:, tq * P:(tq + 1) * P],
                        rhs=w2sb[ot], start=(ot == 0), stop=False)
                emit_out(tq, ps)

# ---------------------------------------------------------------------------
# Host side
# ---------------------------------------------------------------------------

def _hi_lo(b):
    """Split fp32 row vector into bf16 hi + lo rows (hi + lo ~= b in fp32)."""
    import ml_dtypes
    b = np.asarray(b, np.float32)
    hi = b.astype(ml_dtypes.bfloat16)
    lo = (b - hi.astype(np.float32)).astype(ml_dtypes.bfloat16)
    return np.ascontiguousarray(np.stack([hi, lo], axis=0))


def prepare_inputs(x, qkv_w, proj_w, proj_b, rpb_table, n1_w, n1_b, n2_w, n2_b,
                   fc1_w, fc1_b, fc2_w, fc2_b):
    """Fold LN affines into weights, pre-transpose, build shifted exp-bias
    blocks, and produce the 8 per-core input maps."""
    import ml_dtypes
    f = np.float32
    bf = ml_dtypes.bfloat16
    x = np.asarray(x, f)
    qkv_w = np.asarray(qkv_w, f)
    proj_w = np.asarray(proj_w, f)
    proj_b = np.asarray(proj_b, f)
    rpb = np.asarray(rpb_table, f)
    fc1_w = np.asarray(fc1_w, f)
    fc2_w = np.asarray(fc2_w, f)
    n1_w = np.asarray(n1_w, f); n1_b = np.asarray(n1_b, f)
    n2_w = np.asarray(n2_w, f); n2_b = np.asarray(n2_b, f)

    # (g, i) head-packing permutation for the fp8 DoubleRow scores layout:
    # o-tile (g, i) row r holds c_out = 64*(4g + r//32) + 32*i + r%32.
    perm = np.empty(C, np.int64)
    idx = 0
    for g in range(2):
        for i in range(2):
            for r in range(P):
                perm[idx] = DH * (4 * g + r // 32) + 32 * i + r % 32
                idx += 1
    wqkv_s = qkv_w * n1_w[None, :]
    wqkv_p = np.concatenate(
        [wqkv_s[0:C][perm], wqkv_s[C:2 * C][perm], wqkv_s[2 * C:]], axis=0)
    wqkvT = np.ascontiguousarray(wqkv_p.T.astype(bf))
    bqkv = (qkv_w @ n1_b).astype(f)
    bv = bqkv[2 * C:]
    bqk_p = np.concatenate([bqkv[0:C][perm], bqkv[C:2 * C][perm]])
    wprojT = np.ascontiguousarray(proj_w.T.astype(bf))
    # V bias rides through the softmax average: fold bv@proj_w.T into proj_b
    bproj_eff = (proj_b + proj_w @ bv).astype(f)
    wfc1T = np.ascontiguousarray((fc1_w * n2_w[None, :]).T.astype(bf))
    bfc1x = (np.asarray(fc1_b, f) + fc1_w @ n2_b).astype(f)
    wfc2T = np.ascontiguousarray(fc2_w.T.astype(bf))

    # exp-bias blocks: value at (k-tile kt, partition p, own-query j) must be
    # exp(rpb[k_glob - q_glob + N-1, h]); with own-first rolled rows and the
    # view i = j + (7 - kt%8)*128,
    #   half A (kt 0..7):  idx = 2943 + p - i
    #   half B (kt 8..15): idx = 3967 - 2048*parity + p - i
    ii = np.arange(BLKW)[None, :]
    pp = np.arange(P)[:, None]
    idx_a = 2943 + pp - ii
    eblka_np = np.ascontiguousarray(
        np.exp(rpb[idx_a, :]).transpose(2, 0, 1).astype(bf))
    eblkb_np = []
    for par in range(2):
        idx_b = 3967 - 2048 * par + pp - ii
        eblkb_np.append(np.ascontiguousarray(
            np.exp(rpb[idx_b, :]).transpose(2, 0, 1).astype(bf)))

    shared = dict(
        wqkvT=wqkvT,
        bqk=np.ascontiguousarray(bqk_p),
        wprojT=wprojT,
        bproj2=_hi_lo(bproj_eff),
        wfc1T=wfc1T, bfc1=bfc1x, wfc2T=wfc2T,
        bfc22=_hi_lo(fc2_b),
        eblka=eblka_np,
    )
    in_maps = []
    for core in range(8):
        b, par = core // 2, core % 2
        xb_c = np.ascontiguousarray(np.roll(x[b], -par * NQ, axis=0))
        m = dict(shared)
        m["xb"] = xb_c
        m["eblkb"] = eblkb_np[par]
        in_maps.append(m)
    return in_maps


def assemble_output(results):
    out = np.empty((B, N, C), np.float32)
    for core in range(8):
        b, par = core // 2, core % 2
        out[b, par * NQ:(par + 1) * NQ, :] = results[core]["out"]
    return out


_cache = threading.local()


def _get_program():
    nc = getattr(_cache, "nc", None)
    if nc is None:
        nc = build_program(reps=1)
        _cache.nc = nc
    return nc


def kernel(**inputs) -> np.ndarray:
    in_maps = prepare_inputs(**inputs)
    nc = _get_program()
    res = run_bass_kernel_spmd(nc, in_maps, list(range(8)))
    return assemble_output(res.results)


if __name__ == "__main__":
    rng = np.random.default_rng(0)
    ins = {
        "x": rng.standard_normal((B, N, C)).astype(np.float32),
        "qkv_w": (rng.standard_normal((3 * C, C)) * 0.02).astype(np.float32),
        "proj_w": (rng.standard_normal((C, C)) * 0.02).astype(np.float32),
        "proj_b": np.zeros(C, np.float32),
        "rpb_table": (rng.standard_normal((2 * N - 1, H)) * 0.02).astype(np.float32),
        "n1_w": np.ones(C, np.float32), "n1_b": np.zeros(C, np.float32),
        "n2_w": np.ones(C, np.float32), "n2_b": np.zeros(C, np.float32),
        "fc1_w": (rng.standard_normal((HID, C)) * 0.02).astype(np.float32),
        "fc1_b": rng.standard_normal(HID).astype(np.float32),
        "fc2_w": (rng.standard_normal((C, HID)) * 0.02).astype(np.float32),
        "fc2_b": rng.standard_normal(C).astype(np.float32),
    }
    out = kernel(**ins)
    print("out", out.shape, out.dtype, float(np.abs(out).mean()))



# revision 23
# speedup vs baseline: 1.5765x; 1.5765x over previous
"""Trainium2 Bass kernel for a dense transformer block.

Block: y = x + proj(MHA(LN1(x), rel-pos-bias)) ; out = y + fc2(gelu(fc1(LN2(y))))
Shapes (hardcoded): B=4, N=2048, C=512, H=8, DH=64, HID=2048, fp32 I/O.

Sharding over 8 cores: core c -> (batch b = c//2, query-half par = c%2).
Each core receives its batch's rows rolled so its own 1024 query tokens come
first, computes K/V over all 2048 tokens (duplicated across the pair of cores
sharing a batch -- cheaper than a collective), and runs attention + MLP for its
own 1024 tokens. Weights are replicated; LayerNorm affine params are folded
into the matmul weights on the host.

Engine-balance design (v2):
  - softmax bias enters MULTIPLICATIVELY: exp(s+b) = exp(s)*exp(b).  The ACT
    engine exps score PSUM directly ([128,2048] spanning 4 banks covers both
    heads of a pair in one instruction); host supplies exp(bias) blocks in
    bf16; the bias application is then a bf16*bf16 SBUF DVE multiply which
    runs in the DVE's 4x perf mode.
  - LN rsqrt = exp(-0.5*ln(var+eps)) with var columns batched across tiles,
    so the only ACT table sets used are natural_log_exp (A/C/D) and gelu (E).
  - fc1 bias rides the Gelu activation's per-partition bias operand (ACT
    evicts the fc1 PSUM directly); the V bias is folded through the
    attention-average into proj_b on the host; fc2/proj biases enter via a
    K=2 ones-matmul with hi+lo bf16 rows.
  - scores matmuls are head-paired via PE row tiling (K=64 each, partitions
    0-63 / 64-127 -> tile_position (0,0)/(64,0) auto-derived), attnV uses the
    ones-augmented V (M=65) so the softmax denominator accumulates in row 64.
  - O^T stays in SBUF (no DRAM roundtrip); transpose evictions are merged
    into single 512-wide strided copies.
"""

import threading
from contextlib import ExitStack

import numpy as np

import concourse.bass as bass
import concourse.tile as tile
from concourse import bacc, mybir
from concourse.bass_utils import run_bass_kernel_spmd
from concourse.masks import make_identity

F32 = mybir.dt.float32
BF16 = mybir.dt.bfloat16
FP8 = mybir.dt.float8e4

B, N, C, H = 4, 2048, 512, 8
DH = C // H          # 64
HID = 4 * C          # 2048
NQ = N // 2          # own query tokens per core (1024)
EPS = 1e-5
P = 128              # partitions
TT = N // P          # 16 token tiles (full batch)
TQ = NQ // P         # 8 token tiles (own)
CT = C // P          # 4 channel tiles
OT = HID // P        # 16 hidden tiles
BLKW = NQ + 7 * P    # 1920, bias block width


def build_program(reps: int = 1, phases: str = "abcde"):
    """Build the per-core Bass program (SPMD; all per-core differences are
    carried by input data)."""
    nc = bacc.Bacc("TRN2", target_bir_lowering=False, debug=False, num_devices=8)

    t = {}
    t["xb"] = nc.dram_tensor("xb", [N, C], F32, kind="ExternalInput").ap()
    t["wqkvT"] = nc.dram_tensor("wqkvT", [C, 3 * C], BF16,
                                kind="ExternalInput").ap()
    t["bqk"] = nc.dram_tensor("bqk", [2 * C], F32, kind="ExternalInput").ap()
    t["wprojT"] = nc.dram_tensor("wprojT", [C, C], BF16,
                                 kind="ExternalInput").ap()
    t["bproj2"] = nc.dram_tensor("bproj2", [2, C], BF16,
                                 kind="ExternalInput").ap()
    t["wfc1T"] = nc.dram_tensor("wfc1T", [C, HID], BF16,
                                kind="ExternalInput").ap()
    t["bfc1"] = nc.dram_tensor("bfc1", [HID], F32, kind="ExternalInput").ap()
    t["wfc2T"] = nc.dram_tensor("wfc2T", [HID, C], BF16,
                                kind="ExternalInput").ap()
    t["bfc22"] = nc.dram_tensor("bfc22", [2, C], BF16,
                                kind="ExternalInput").ap()
    t["eblka"] = nc.dram_tensor("eblka", [H, P, BLKW], BF16,
                                kind="ExternalInput").ap()
    t["eblkb"] = nc.dram_tensor("eblkb", [H, P, BLKW], BF16,
                                kind="ExternalInput").ap()
    t["out"] = nc.dram_tensor("out", [NQ, C], F32, kind="ExternalOutput").ap()

    with tile.TileContext(nc) as tc:
        if reps == 1:
            _build_body(nc, tc, t)
        else:
            with tc.For_i(0, reps, 1):
                _build_body(nc, tc, t)
    nc.compile()
    return nc


def _build_body(nc, tc, t):
    Act = mybir.ActivationFunctionType
    Alu = mybir.AluOpType

    xb, out = t["xb"], t["out"]

    with ExitStack() as ctx:
        singles = ctx.enter_context(tc.tile_pool(name="singles", bufs=1))
        ident = singles.tile([P, P], F32)
        make_identity(nc, ident)
        identB = singles.tile([P, P], BF16)
        nc.vector.tensor_copy(out=identB, in_=ident)
        eps_t = singles.tile([P, 1], F32)
        nc.gpsimd.memset(eps_t, EPS)
        ones2 = singles.tile([2, P], BF16)
        nc.gpsimd.memset(ones2, 1.0)
        ones64f = singles.tile([1, DH], F32)
        nc.gpsimd.memset(ones64f, 1.0)

        x_all = [None] * TT
        kT8 = [None] * 2     # [P, 2*N] fp8: heads 4g+j at parts 32j, dh-half
        qT8 = [None] * 2     # [P, 2*NQ] fp8, same packing
        kTh = [None] * H     # [32, 2*N] fp8 per head at partitions 0:32
        qTh = [None] * H     # [32, 2*NQ] fp8 per head (DR tiles at (0,0):
        # nonzero-row-position DoubleRow matmuls fault on TRN2 hw, so the
        # packed evictions are DMA-remapped down to partition 0 per head)
        va = [None] * TT

        xq_pool = ctx.enter_context(tc.tile_pool(name="xq", bufs=TQ))
        oT_pool = ctx.enter_context(tc.tile_pool(name="oT", bufs=CT))
        # D/E weights live here so they can prefetch during phase C while
        # phase-C pools (created later) still release first (LIFO).
        wts_pool = ctx.enter_context(tc.tile_pool(name="wts", bufs=1))
        ac_scope = ctx.enter_context(ExitStack())  # spans phases A..C
        kT_pool = ac_scope.enter_context(tc.tile_pool(name="kT", bufs=CT))
        qT_pool = ac_scope.enter_context(tc.tile_pool(name="qT", bufs=CT))
        va_pool = ac_scope.enter_context(tc.tile_pool(name="va", bufs=TT))
        # exp-bias block pool outlives the A/B scope (prefetched during B)
        eb_pool = ac_scope.enter_context(tc.tile_pool(name="eb", bufs=4))

        # O^T in SBUF: oT[hp] is [128, NQ] holding heads 2hp (rows 0:64) and
        # 2hp+1 (rows 64:128) -- exactly the proj lhsT channel tile.
        oT = [oT_pool.tile([P, NQ], BF16, tag="oT", name=f"oT{i}")
              for i in range(CT)]

        # ------------------------------------------------------------------
        # Phases A+B+C, software-pipelined: LN1 is processed in two token
        # halves; QKV chunks for heads 2.. are emitted inside the attention
        # kt-loops of earlier head-pairs (PE has slack under the exp period).
        # ------------------------------------------------------------------
        ab = ac_scope.enter_context(ExitStack())
        z1t_pool = ab.enter_context(tc.tile_pool(name="z1t", bufs=1))
        xload_pool = ab.enter_context(
            tc.tile_pool(name="xload", bufs=TT - TQ))
        zt_pool = ab.enter_context(tc.tile_pool(name="zt", bufs=3))
        stat_pool = ab.enter_context(tc.tile_pool(name="stat", bufs=4))
        mv_pool = ab.enter_context(tc.tile_pool(name="mv1", bufs=1))
        wq_pool = ab.enter_context(tc.tile_pool(name="wq", bufs=CT))
        bias_pool = ab.enter_context(tc.tile_pool(name="qkvb", bufs=1))

        # z1t split into token halves so QKV can start after half A
        z1h = []      # z1h[half][ct] = [P, NQ] view
        z1c = []
        for half in range(2):
            z_all = z1t_pool.tile([P, CT * NQ], BF16, tag=f"z1t{half}",
                                  name=f"z1t{half}")
            z1h.append([z_all[:, ct * NQ:(ct + 1) * NQ] for ct in range(CT)])
            z1c.append(z_all.rearrange("p (c n) -> p c n", c=CT))

        mv_all = mv_pool.tile([P, 2 * TT], F32, tag="mv")
        lnv = mv_pool.tile([P, TT], F32, tag="lnv")
        rs_all = mv_pool.tile([P, TT], F32, tag="rs")
        mv_t = mv_all.rearrange("p (t two) -> p t two", two=2)
        lnv_t = lnv.rearrange("p (t one) -> p t one", one=1)

        with ExitStack() as abp:
            tpsum = abp.enter_context(
                tc.tile_pool(name="tpsum", bufs=2, space="PSUM"))
            bpsum = abp.enter_context(
                tc.tile_pool(name="bpsum", bufs=4, space="PSUM"))

            def ln1_quarter(qb):
                t0, t1 = qb * 4, qb * 4 + 4
                for tt in range(t0, t1):
                    if tt < TQ:
                        x_t = xq_pool.tile([P, C], F32, tag="xq")
                    else:
                        x_t = xload_pool.tile([P, C], F32, tag="xload")
                    x_all[tt] = x_t
                    nc.sync.dma_start(out=x_t, in_=xb[tt * P:(tt + 1) * P, :])
                    st = stat_pool.tile([P, 6], F32, tag="st")
                    nc.vector.bn_stats(out=st, in_=x_t)
                    nc.vector.bn_aggr(out=mv_all[:, 2 * tt:2 * tt + 2],
                                      in_=st)
                # rs = exp(-0.5*ln(var+eps)), batched over the half-batch
                nc.scalar.activation(out=lnv_t[:, t0:t1, :],
                                     in_=mv_t[:, t0:t1, 1:2],
                                     func=Act.Ln, bias=eps_t, scale=1.0)
                nc.scalar.activation(out=rs_all[:, t0:t1],
                                     in_=lnv[:, t0:t1],
                                     func=Act.Exp, scale=-0.5)
                for tt in range(t0, t1):
                    z_t = zt_pool.tile([P, C], BF16, tag="zt")
                    nc.vector.tensor_scalar(
                        out=z_t, in0=x_all[tt],
                        scalar1=mv_all[:, 2 * tt:2 * tt + 1],
                        scalar2=rs_all[:, tt:tt + 1],
                        op0=Alu.subtract, op1=Alu.mult)
                    ps4 = tpsum.tile([P, C], BF16, tag="tr")
                    for ct in range(CT):
                        nc.tensor.transpose(
                            ps4[:, ct * P:(ct + 1) * P],
                            z_t[:, ct * P:(ct + 1) * P], identB)
                    hb, tl = tt // 8, tt % 8
                    nc.vector.tensor_copy(
                        out=z1c[hb][:, :, tl * P:(tl + 1) * P],
                        in_=ps4.rearrange("p (c n) -> p c n", c=CT))

            wsb = []
            bcols = []

            def qkv_weights():
                for g in range(2):
                    kT8[g] = kT_pool.tile([P, 2 * N], FP8, tag="kT",
                                          name=f"kT{g}", bufs=2)
                    qT8[g] = qT_pool.tile([P, 2 * NQ], FP8, tag="qT",
                                          name=f"qT{g}", bufs=2)
                for ct in range(CT):
                    w_t = wq_pool.tile([P, 3 * C], BF16, tag="wq")
                    nc.sync.dma_start(
                        out=w_t, in_=t["wqkvT"][ct * P:(ct + 1) * P, :])
                    wsb.append(w_t)
                for ot in range(8):
                    bt = bias_pool.tile([P, 1], F32, tag="bcol", bufs=8)
                    nc.sync.dma_start(
                        out=bt,
                        in_=t["bqk"][ot * P:(ot + 1) * P].rearrange(
                            "(p one) -> p one", one=1))
                    bcols.append(bt)

            def v_tile(tt):
                # V natural [tok, 512] + ones column per head -> [P, H, 65]
                hb, tl = tt // 8, tt % 8
                v_t = va_pool.tile([P, H * (DH + 1)], BF16, tag="va")
                va[tt] = v_t
                nc.gpsimd.memset(v_t, 1.0)
                ps = bpsum.tile([P, 512], F32, tag="mm")
                for ct in range(CT):
                    nc.tensor.matmul(
                        ps,
                        lhsT=z1h[hb][ct][:, tl * P:(tl + 1) * P],
                        rhs=wsb[ct][:, 2 * C:3 * C],
                        start=(ct == 0), stop=(ct == CT - 1))
                nc.vector.tensor_copy(
                    out=v_t.rearrange("p (h w) -> p h w",
                                      w=DH + 1)[:, :, 0:DH],
                    in_=ps.rearrange("p (h w) -> p h w", w=DH))

            def k_chunk(ot, tch):
                # K^T o-tile ot=(g,i), 512 tokens at tch*512; fp8 eviction
                # into the DoubleRow dh-split layout.
                g, i = ot // 2, ot % 2
                hb, tl = tch // 2, tch % 2
                ps = bpsum.tile([P, 512], F32, tag="mm")
                for ct in range(CT):
                    nc.tensor.matmul(
                        ps,
                        lhsT=wsb[ct][:, C + ot * P:C + (ot + 1) * P],
                        rhs=z1h[hb][ct][:, tl * 512:(tl + 1) * 512],
                        start=(ct == 0), stop=(ct == CT - 1))
                nc.vector.tensor_scalar_add(
                    out=kT8[g][:, i * N + tch * 512:i * N + (tch + 1) * 512],
                    in0=ps, scalar1=bcols[4 + ot])

            def q_chunk(ot, tch):
                # Q^T o-tile ot=(g,i), own tokens only (token half A)
                g, i = ot // 2, ot % 2
                ps = bpsum.tile([P, 512], F32, tag="mm")
                for ct in range(CT):
                    nc.tensor.matmul(
                        ps,
                        lhsT=wsb[ct][:, ot * P:(ot + 1) * P],
                        rhs=z1h[0][ct][:, tch * 512:(tch + 1) * 512],
                        start=(ct == 0), stop=(ct == CT - 1))
                nc.vector.tensor_scalar_add(
                    out=qT8[g][:, i * NQ + tch * 512:i * NQ + (tch + 1) * 512],
                    in0=ps, scalar1=bcols[ot])

            def qkv_head_chunks(ot):
                return ([("k", ot, tch) for tch in range(4)]
                        + [("q", ot, tch) for tch in range(2)])

            def emit_chunk(c):
                kind = c[0]
                if kind == "k":
                    k_chunk(c[1], c[2])
                elif kind == "q":
                    q_chunk(c[1], c[2])
                elif kind == "v":
                    v_tile(c[1])

            # exp-bias block loads ride the ACT hwdge queue so they stream
            # in parallel with the x/weight loads on the SP queue.
            eb_all = [None] * H

            def eb_load(h):
                bb = eb_pool.tile([P, 2 * BLKW], BF16, tag="eb")
                nc.scalar.dma_start(out=bb[:, 0:BLKW], in_=t["eblka"][h])
                nc.scalar.dma_start(out=bb[:, BLKW:2 * BLKW],
                                    in_=t["eblkb"][h])
                return bb

            # --- emission: A quarters; heads-0-3 QKV first; V; rest ---
            ln1_quarter(0)
            ln1_quarter(1)
            qkv_weights()
            for ot in range(2):
                for tch in range(2):
                    k_chunk(ot, tch)
            for ot in range(2):
                for tch in range(2):
                    q_chunk(ot, tch)
            for tt in range(8):
                v_tile(tt)
            ln1_quarter(2)
            ln1_quarter(3)
            eb_all[0] = eb_load(0)
            for ot in range(2):
                for tch in range(2, 4):
                    k_chunk(ot, tch)
            for tt in range(8, 16):
                v_tile(tt)
            eb_all[1] = eb_load(1)

            def head_remap(h):
                g, j = h // 4, h % 4
                kTh[h] = kT_pool.tile([32, 2 * N], FP8, tag="kTh",
                                      name=f"kTh{h}", bufs=4)
                qTh[h] = qT_pool.tile([32, 2 * NQ], FP8, tag="qTh",
                                      name=f"qTh{h}", bufs=4)
                nc.sync.dma_start(out=kTh[h],
                                  in_=kT8[g][32 * j:32 * (j + 1), :])
                nc.sync.dma_start(out=qTh[h],
                                  in_=qT8[g][32 * j:32 * (j + 1), :])

            for h in range(4):
                head_remap(h)
            # remaining QKV o-tiles (heads 4-7; PSUM is full during
            # attention, so these are emitted up front)
            for ot in range(2, 4):
                for tch in range(4):
                    k_chunk(ot, tch)
                for tch in range(2):
                    q_chunk(ot, tch)
            for h in range(4, H):
                head_remap(h)
            # D/E weight prefetch on the (now idle) SP queue
            wpsb = []
            for ct in range(CT):
                w_t = wts_pool.tile([P, C], BF16, tag="wp", bufs=CT)
                nc.sync.dma_start(out=w_t,
                                  in_=t["wprojT"][ct * P:(ct + 1) * P, :])
                wpsb.append(w_t)
            bprow = wts_pool.tile([2, C], BF16, tag="bprow")
            nc.sync.dma_start(out=bprow, in_=t["bproj2"])
            w1sb = []
            for ct in range(CT):
                w_t = wts_pool.tile([P, HID], BF16, tag="w1", bufs=CT)
                nc.sync.dma_start(out=w_t,
                                  in_=t["wfc1T"][ct * P:(ct + 1) * P, :])
                w1sb.append(w_t)
            w2sb = []
            for ot in range(OT):
                w_t = wts_pool.tile([P, C], BF16, tag="w2", bufs=OT)
                nc.sync.dma_start(out=w_t,
                                  in_=t["wfc2T"][ot * P:(ot + 1) * P, :])
                w2sb.append(w_t)
            b1cols = []
            for ot in range(OT):
                bt = wts_pool.tile([P, 1], F32, tag="b1c", bufs=OT)
                nc.sync.dma_start(
                    out=bt, in_=t["bfc1"][ot * P:(ot + 1) * P].rearrange(
                        "(p one) -> p one", one=1))
                b1cols.append(bt)
            b2row = wts_pool.tile([2, C], BF16, tag="b2row")
            nc.sync.dma_start(out=b2row, in_=t["bfc22"])
            abp.close()   # free tpsum/bpsum banks for attention
            ab.close()    # free z1t/xload/wq SBUF before attention tiles

            # --------------------------------------------------------------
            # Phase C: attention, head-pairs, flash-style over k tiles.
            # Per (kt, head): scores -> one [128,1024] 2-bank PSUM chunk,
            # one Exp -> bf16, one 4x-mode DVE multiply by exp(bias).
            # attnV uses ones-augmented V (M=65): denominator in row 64.
            # The 1/den broadcast lands in partitions 64:128 of the (full
            # height) ac tile, so normalize needs no extra PSUM.
            # --------------------------------------------------------------
            cx = ac_scope.enter_context(ExitStack())
            texp_pool = cx.enter_context(tc.tile_pool(name="texp", bufs=4))
            traw_pool = cx.enter_context(tc.tile_pool(name="traw", bufs=3))
            rden_pool = cx.enter_context(tc.tile_pool(name="rden", bufs=2))
            scp = cx.enter_context(
                tc.tile_pool(name="scp", bufs=3, space="PSUM"))
            acp = cx.enter_context(
                tc.tile_pool(name="acp", bufs=1, space="PSUM"))

            def normalize(h, ac):
                # Evacuate num+den to SBUF immediately (frees the single ac
                # PSUM buffer for the next head), then 1/den on DVE and the
                # broadcast+multiply on the otherwise-idle GPSIMD engine.
                den_sb = rden_pool.tile([1, NQ], F32, tag="densb", bufs=2)
                nc.vector.tensor_copy(out=den_sb, in_=ac[DH:DH + 1, :])
                num_sb = rden_pool.tile([DH, NQ], BF16, tag="numsb", bufs=2)
                nc.vector.tensor_copy(out=num_sb, in_=ac[0:DH, :])
                rden = rden_pool.tile([1, NQ], F32, tag="rden")
                nc.vector.reciprocal_approx_fast(out=rden, in_=den_sb)
                rb = rden_pool.tile([DH, NQ], F32, tag="rb", bufs=2)
                nc.gpsimd.partition_broadcast(out_ap=rb, in_ap=rden)
                nc.gpsimd.tensor_tensor(
                    out=oT[h // 2][(h % 2) * DH:(h % 2 + 1) * DH, :],
                    in0=num_sb, in1=rb, op=Alu.mult)

            DRmode = mybir.MatmulPerfMode.DoubleRow
            pend_norm = None
            for h in range(H):
                g, j = h // 4, h % 4
                if h + 2 < H:
                    eb_all[h + 2] = eb_load(h + 2)
                bb = eb_all[h]
                kTv = kTh[h].rearrange("p (two n) -> p two n", two=2)
                qTv = qTh[h].rearrange("p (two n) -> p two n", two=2)
                ac = acp.tile([P, NQ], F32, tag="ac", name=f"ac{h}")

                def attnv(ptx, pkt):
                    for qc in range(2):
                        nc.tensor.matmul(
                            ac[0:DH + 1, qc * 512:(qc + 1) * 512],
                            lhsT=va[pkt][:, h * (DH + 1):(h + 1) * (DH + 1)],
                            rhs=ptx[:, qc * 512:(qc + 1) * 512],
                            start=(pkt == 0), stop=(pkt == TT - 1))

                # attnV trails scores by PENDD kt so the PE never waits on
                # the exp->mult chain (the TRN2 PE p-state needs a gapless
                # stream to ramp to full clock).
                PENDD = 3
                pend = []
                for kt in range(TT):
                    off = (0 if kt < 8 else BLKW) + (7 - kt % 8) * P
                    scl = scp.tile([P, NQ], F32, tag="sc",
                                   name=f"sc{h}_{kt}")
                    for qc in range(2):
                        nc.tensor.matmul(
                            scl[:, qc * 512:(qc + 1) * 512],
                            lhsT=kTv[:, :, kt * P:(kt + 1) * P],
                            rhs=qTv[:, :, qc * 512:(qc + 1) * 512],
                            start=True, stop=True, perf_mode=DRmode,
                            tile_position=(0, 0))
                    traw = traw_pool.tile([P, NQ], BF16, tag="traw")
                    nc.scalar.activation(out=traw, in_=scl, func=Act.Exp,
                                         scale=float(DH) ** -0.5)
                    tx = texp_pool.tile([P, NQ], BF16, tag="tx",
                                        name=f"tx{h}_{kt}")
                    nc.vector.tensor_tensor(
                        out=tx, in0=traw, in1=bb[:, off:off + NQ],
                        op=Alu.mult)
                    if kt == 1 and pend_norm is not None:
                        normalize(*pend_norm)
                        pend_norm = None
                    pend.append((tx, kt))
                    if len(pend) > PENDD:
                        attnv(*pend.pop(0))
                for ptx, pkt in pend:
                    attnv(ptx, pkt)
                pend_norm = (h, ac)
            normalize(*pend_norm)
        ac_scope.close()  # free kT/qT/va/z1t/eb/texp + attention PSUM

        # ------------------------------------------------------------------
        # Phase D: proj + residual + LN2 (transposed), two batches of 4
        # ------------------------------------------------------------------
        x2_pool = ctx.enter_context(tc.tile_pool(name="x2", bufs=TQ))
        z2t_pool = ctx.enter_context(tc.tile_pool(name="z2t", bufs=1))
        z2t_all = z2t_pool.tile([P, CT * NQ], BF16, tag="z2t")
        z2t = [z2t_all[:, ct * NQ:(ct + 1) * NQ] for ct in range(CT)]
        z2t_c = z2t_all.rearrange("p (c n) -> p c n", c=CT)
        x2 = [None] * TQ
        with ExitStack() as dx:
            stat2_pool = dx.enter_context(tc.tile_pool(name="stat2", bufs=4))
            mv2_pool = dx.enter_context(tc.tile_pool(name="mv2", bufs=1))
            zt2_pool = dx.enter_context(tc.tile_pool(name="zt2", bufs=3))
            dpsum = dx.enter_context(
                tc.tile_pool(name="dpsum", bufs=2, space="PSUM"))
            tpsum2 = dx.enter_context(
                tc.tile_pool(name="tpsum2", bufs=2, space="PSUM"))

            mv2_all = mv2_pool.tile([P, 2 * TQ], F32, tag="mv2")
            lnv2 = mv2_pool.tile([P, TQ], F32, tag="lnv2")
            rs2_all = mv2_pool.tile([P, TQ], F32, tag="rs2")
            mv2_t = mv2_all.rearrange("p (t two) -> p t two", two=2)
            lnv2_t = lnv2.rearrange("p (t one) -> p t one", one=1)

            for hb in range(2):
                t0, t1 = hb * 4, hb * 4 + 4
                for tq in range(t0, t1):
                    ps = dpsum.tile([P, C], F32, tag="mm")
                    for ct in range(CT):
                        nc.tensor.matmul(
                            ps, lhsT=oT[ct][:, tq * P:(tq + 1) * P],
                            rhs=wpsb[ct], start=(ct == 0), stop=False)
                    nc.tensor.matmul(ps, lhsT=ones2, rhs=bprow,
                                     start=False, stop=True)
                    x2_t = x2_pool.tile([P, C], F32, tag="x2")
                    x2[tq] = x2_t
                    nc.vector.tensor_add(out=x2_t, in0=ps, in1=x_all[tq])
                    st = stat2_pool.tile([P, 6], F32, tag="st2")
                    nc.vector.bn_stats(out=st, in_=x2_t)
                    nc.vector.bn_aggr(out=mv2_all[:, 2 * tq:2 * tq + 2],
                                      in_=st)
                nc.scalar.activation(out=lnv2_t[:, t0:t1, :],
                                     in_=mv2_t[:, t0:t1, 1:2],
                                     func=Act.Ln, bias=eps_t, scale=1.0)
                nc.scalar.activation(out=rs2_all[:, t0:t1],
                                     in_=lnv2[:, t0:t1],
                                     func=Act.Exp, scale=-0.5)
                for tq in range(t0, t1):
                    z_t = zt2_pool.tile([P, C], BF16, tag="z2tmp")
                    nc.vector.tensor_scalar(
                        out=z_t, in0=x2[tq],
                        scalar1=mv2_all[:, 2 * tq:2 * tq + 1],
                        scalar2=rs2_all[:, tq:tq + 1],
                        op0=Alu.subtract, op1=Alu.mult)
                    ps4 = tpsum2.tile([P, C], BF16, tag="tr2")
                    for ct in range(CT):
                        nc.tensor.transpose(
                            ps4[:, ct * P:(ct + 1) * P],
                            z_t[:, ct * P:(ct + 1) * P], identB)
                    nc.vector.tensor_copy(
                        out=z2t_c[:, :, tq * P:(tq + 1) * P],
                        in_=ps4.rearrange("p (c n) -> p c n", c=CT))

        # ------------------------------------------------------------------
        # Phase E: MLP.  fc1 bias rides the Gelu bias operand (ACT evicts
        # the fc1 PSUM); fc2 runs ot-outer into 6 resident accumulators so
        # its matmuls interleave with fc1's instead of waiting for all of
        # gelu -- only tq 6..7 run as a short tail.
        # ------------------------------------------------------------------
        with ExitStack() as ex:
            g_pool = ex.enter_context(tc.tile_pool(name="g", bufs=3))
            out_pool = ex.enter_context(tc.tile_pool(name="outp", bufs=2))
            gpsum = ex.enter_context(
                tc.tile_pool(name="gpsum", bufs=2, space="PSUM"))
            epsum = ex.enter_context(
                tc.tile_pool(name="epsum", bufs=6, space="PSUM"))

            NACC = 6

            def emit_out(tq, ps):
                nc.tensor.matmul(ps, lhsT=ones2, rhs=b2row,
                                 start=False, stop=True)
                o_t = out_pool.tile([P, C], F32, tag="out")
                nc.vector.tensor_add(out=o_t, in0=ps, in1=x2[tq])
                nc.sync.dma_start(out=out[tq * P:(tq + 1) * P, :], in_=o_t)

            eps_acc = [epsum.tile([P, C], F32, tag="mm2", name=f"fc2a{i}")
                       for i in range(NACC)]
            gT = []
            for ot in range(OT):
                g_t = g_pool.tile([P, NQ], BF16, tag="g", bufs=OT)
                gT.append(g_t)
                for qc in range(NQ // 512):
                    psg = gpsum.tile([P, 512], F32, tag="mm1")
                    for ct in range(CT):
                        nc.tensor.matmul(
                            psg,
                            lhsT=w1sb[ct][:, ot * P:(ot + 1) * P],
                            rhs=z2t[ct][:, qc * 512:(qc + 1) * 512],
                            start=(ct == 0), stop=(ct == CT - 1))
                    nc.scalar.activation(
                        out=g_t[:, qc * 512:(qc + 1) * 512], in_=psg,
                        func=Act.Gelu, bias=b1cols[ot], scale=1.0)
                # fc2 partials for tq 0..NACC-1 ride along (one ot behind)
                if ot >= 1:
                    for tq in range(NACC):
                        nc.tensor.matmul(
                            eps_acc[tq],
                            lhsT=gT[ot - 1][:, tq * P:(tq + 1) * P],
                            rhs=w2sb[ot - 1],
                            start=(ot - 1 == 0), stop=False)
            for tq in range(NACC):
                nc.tensor.matmul(
                    eps_acc[tq], lhsT=gT[OT - 1][:, tq * P:(tq + 1) * P],
                    rhs=w2sb[OT - 1], start=False, stop=False)
                emit_out(tq, eps_acc[tq])
            for tq in range(NACC, TQ):
                ps = epsum.tile([P, C], F32, tag="mm2", name=f"fc2t{tq}")
                for ot in range(OT):
                    nc.tensor.matmul(
                        ps, lhsT=gT[ot][:, tq * P:(tq + 1) * P],
                        rhs=w2sb[ot], start=(ot == 0), stop=False)
                emit_out(tq, ps)

# ---------------------------------------------------------------------------
# Host side
# ---------------------------------------------------------------------------

def _hi_lo(b):
    """Split fp32 row vector into bf16 hi + lo rows (hi + lo ~= b in fp32)."""
    import ml_dtypes
    b = np.asarray(b, np.float32)
    hi = b.astype(ml_dtypes.bfloat16)
    lo = (b - hi.astype(np.float32)).astype(ml_dtypes.bfloat16)
    return np.ascontiguousarray(np.stack([hi, lo], axis=0))


def prepare_inputs(x, qkv_w, proj_w, proj_b, rpb_table, n1_w, n1_b, n2_w, n2_b,
                   fc1_w, fc1_b, fc2_w, fc2_b):
    """Fold LN affines into weights, pre-transpose, build shifted exp-bias
    blocks, and produce the 8 per-core input maps."""
    import ml_dtypes
    f = np.float32
    bf = ml_dtypes.bfloat16
    x = np.asarray(x, f)
    qkv_w = np.asarray(qkv_w, f)
    proj_w = np.asarray(proj_w, f)
    proj_b = np.asarray(proj_b, f)
    rpb = np.asarray(rpb_table, f)
    fc1_w = np.asarray(fc1_w, f)
    fc2_w = np.asarray(fc2_w, f)
    n1_w = np.asarray(n1_w, f); n1_b = np.asarray(n1_b, f)
    n2_w = np.asarray(n2_w, f); n2_b = np.asarray(n2_b, f)

    # (g, i) head-packing permutation for the fp8 DoubleRow scores layout:
    # o-tile (g, i) row r holds c_out = 64*(4g + r//32) + 32*i + r%32.
    perm = np.empty(C, np.int64)
    idx = 0
    for g in range(2):
        for i in range(2):
            for r in range(P):
                perm[idx] = DH * (4 * g + r // 32) + 32 * i + r % 32
                idx += 1
    wqkv_s = qkv_w * n1_w[None, :]
    wqkv_p = np.concatenate(
        [wqkv_s[0:C][perm], wqkv_s[C:2 * C][perm], wqkv_s[2 * C:]], axis=0)
    wqkvT = np.ascontiguousarray(wqkv_p.T.astype(bf))
    bqkv = (qkv_w @ n1_b).astype(f)
    bv = bqkv[2 * C:]
    bqk_p = np.concatenate([bqkv[0:C][perm], bqkv[C:2 * C][perm]])
    wprojT = np.ascontiguousarray(proj_w.T.astype(bf))
    # V bias rides through the softmax average: fold bv@proj_w.T into proj_b
    bproj_eff = (proj_b + proj_w @ bv).astype(f)
    wfc1T = np.ascontiguousarray((fc1_w * n2_w[None, :]).T.astype(bf))
    bfc1x = (np.asarray(fc1_b, f) + fc1_w @ n2_b).astype(f)
    wfc2T = np.ascontiguousarray(fc2_w.T.astype(bf))

    # exp-bias blocks: value at (k-tile kt, partition p, own-query j) must be
    # exp(rpb[k_glob - q_glob + N-1, h]); with own-first rolled rows and the
    # view i = j + (7 - kt%8)*128,
    #   half A (kt 0..7):  idx = 2943 + p - i
    #   half B (kt 8..15): idx = 3967 - 2048*parity + p - i
    ii = np.arange(BLKW)[None, :]
    pp = np.arange(P)[:, None]
    idx_a = 2943 + pp - ii
    eblka_np = np.ascontiguousarray(
        np.exp(rpb[idx_a, :]).transpose(2, 0, 1).astype(bf))
    eblkb_np = []
    for par in range(2):
        idx_b = 3967 - 2048 * par + pp - ii
        eblkb_np.append(np.ascontiguousarray(
            np.exp(rpb[idx_b, :]).transpose(2, 0, 1).astype(bf)))

    shared = dict(
        wqkvT=wqkvT,
        bqk=np.ascontiguousarray(bqk_p),
        wprojT=wprojT,
        bproj2=_hi_lo(bproj_eff),
        wfc1T=wfc1T, bfc1=bfc1x, wfc2T=wfc2T,
        bfc22=_hi_lo(fc2_b),
        eblka=eblka_np,
    )
    in_maps = []
    for core in range(8):
        b, par = core // 2, core % 2
        xb_c = np.ascontiguousarray(np.roll(x[b], -par * NQ, axis=0))
        m = dict(shared)
        m["xb"] = xb_c
        m["eblkb"] = eblkb_np[par]
        in_maps.append(m)
    return in_maps


def assemble_output(results):
    out = np.empty((B, N, C), np.float32)
    for core in range(8):
        b, par = core // 2, core % 2
        out[b, par * NQ:(par + 1) * NQ, :] = results[core]["out"]
    return out


_cache = threading.local()


def _get_program():
    nc = getattr(_cache, "nc", None)
    if nc is None:
        nc = build_program(reps=1)
        _cache.nc = nc
    return nc


def kernel(**inputs) -> np.ndarray:
    in_maps = prepare_inputs(**inputs)
    nc = _get_program()
    res = run_bass_kernel_spmd(nc, in_maps, list(range(8)))
    return assemble_output(res.results)


if __name__ == "__main__":
    rng = np.random.default_rng(0)
    ins = {
        "x": rng.standard_normal((B, N, C)).astype(np.float32),
        "qkv_w": (rng.standard_normal((3 * C, C)) * 0.02).astype(np.float32),
        "proj_w": (rng.standard_normal((C, C)) * 0.02).astype(np.float32),
        "proj_b": np.zeros(C, np.float32),
        "rpb_table": (rng.standard_normal((2 * N - 1, H)) * 0.02).astype(np.float32),
        "n1_w": np.ones(C, np.float32), "n1_b": np.zeros(C, np.float32),
        "n2_w": np.ones(C, np.float32), "n2_b": np.zeros(C, np.float32),
        "fc1_w": (rng.standard_normal((HID, C)) * 0.02).astype(np.float32),
        "fc1_b": rng.standard_normal(HID).astype(np.float32),
        "fc2_w": (rng.standard_normal((C, HID)) * 0.02).astype(np.float32),
        "fc2_b": rng.standard_normal(C).astype(np.float32),
    }
    out = kernel(**ins)
    print("out", out.shape, out.dtype, float(np.abs(out).mean()))

